# revision 37
# baseline (speedup 1.0000x reference)
"""Trainium2 Bass kernel for single-head causal attention.

Problem: x[4,2048,1024] f32; Wq/Wk/Wv [1024,1024] (torch Linear layout, y = x@W.T).
  q,k,v = x@W.T ; scores = q@k.T (causal masked, scaled 1/sqrt(1024)) ;
  out = softmax(scores)@v.

Algebra: with M := Wq^T Wk folded on the host, scores = xq M xk^T (the K
projection disappears; x^T itself is the key matrix), and
out = softmax(.) @ x @ Wv^T (the V projection becomes a postmultiply).

Sharding: 2 cores per batch, zig-zag query-block split (core h=0 gets blocks
[0,15,2,13,4,11,6,9], h=1 [1,14,3,12,5,10,7,8]) so all 8 cores run one SPMD
program with identical causal extents CJ=[1,8,2,7,3,6,4,5] (x 256 keys).

Precision: slots 1..7 (blocks 2..15, causal extent >= 512) run the entire
matmul chain in fp8-e4m3 with DoubleRow perf mode (2 contraction tiles per
instruction, ~1.8x PE throughput); slot 0 (blocks 0/1, peaked attention where
fp8 noise concentrates) stays bf16. Verified error ~0.7% vs the 2% gate.
Scale bookkeeping (powers of 2 throughout): x*16, M*512, Wv^T*2048 on host;
qMT8 = qM_psum/128; QK psum = 1024*scores -> exp scale 2^-15 with bias ln16
so wts = 16*exp(s/32); pass-A psum * (1/accum) = 16*normalized(w)@x; pass-B
psum * 2^-15 = out. No max-subtraction (|scores|/32 < 2, exp cannot overflow).

Structure per core (single rotating 2-bank PSUM pool spans all phases so
phase boundaries never wait on a full pool handoff):
  warmup MMs (HAM un-throttle, sized to the input-DMA arrival) ->
  qM: fp8-DR for query cols 0:896 (slots 1-7) + bf16 for cols 896:1024
      (slot 0); queries are column-reordered so the fp8 slots are contiguous ->
  QK per slot (big/small order, slot 0 mid): dp-outer DR MMs into <=1024-key
      PSUM chunks, causal edge mask added in-PSUM by DVE, exp straight from
      PSUM per chunk (ACT, accum_out = row sum), reciprocal on DVE ->
  pass A: ALL weight transposes run on the PE itself, grouped ahead of the
      w@x matmuls (the PE would otherwise idle while the input-DMA queue
      drains, and a grouped block avoids interleaving transpose-mode
      LDWEIGHTS with in-flight DR matmuls, which corrupted weights when
      tried inline); bf16-psum pair copies emit fp8 directly. DR MMs w @ x
      accumulate over key blocks; the PSUM->SBUF copy applies 1/sum ->
  wx_sb --DMA-transpose--> wxT -> fp8 (DVE/ACT alternating) ->
  pass B: DR MMs (16*wbar x) @ (2048*Wv^T), scaled bf16 out, DMA out.
Queues: all inputs stream on the sync HWDGE queue in first-use order; all
xbar transposes also ride the sync queue (two queues would race the xbar);
output DMAs fill the sync queue tail.
"""

from contextlib import ExitStack

import ml_dtypes
import numpy as np

import concourse.mybir as mybir
import concourse.tile as tile
from concourse import bacc
from concourse.bass_utils import run_bass_kernel_spmd

B, S, D, E = 4, 2048, 1024, 1024
P = 128
N_CORES = 8
DT = D // P          # 8 contraction tiles
SQ = S // 2          # 1024 query rows per core
KCH = 256            # causal-length granularity
NSLOT = SQ // P      # 8 query slots per core

QBLOCKS = [[0, 15, 2, 13, 4, 11, 6, 9], [1, 14, 3, 12, 5, 10, 7, 8]]
CJ = [(b + 2) // 2 for b in QBLOCKS[0]]       # [1,8,2,7,3,6,4,5]
assert CJ == [(b + 2) // 2 for b in QBLOCKS[1]]
# fp8 slots 1..7 at query columns (j-1)*128; bf16 slot 0 at columns 896:1024
QCOL = [7 * P] + [(j - 1) * P for j in range(1, NSLOT)]
SLOT_ORDER = [1, 3, 5, 0, 7, 6, 4, 2]  # big first; bf16 slot 0 mid; small last

F32 = mybir.dt.float32
BF16 = mybir.dt.bfloat16
FP8 = mybir.dt.float8e4
DR = mybir.MatmulPerfMode.DoubleRow
EXP = mybir.ActivationFunctionType.Exp
LN16 = float(np.log(16.0))
MASK_VAL = -1.0e9

S_X = 16.0           # fp8 scale for x-derived tensors
S_M = 512.0          # fp8 scale for M
S_V = 2048.0         # fp8 scale for Wv^T
C_Q = 1.0 / 128.0    # qM psum -> qMT8 copy scale (qMT8 = 64*qM_true)
AS_F8 = 1.0 / 32768.0   # QK psum scale for fp8 slots: (1/32) / 1024
AS_BF = 1.0 / 32.0      # slot-0 QK psum is true-scale
OUT_F8 = 1.0 / 32768.0  # pass-B psum scale, fp8 slots: 1/(16*2048)
OUT_BF = 1.0 / 16.0     # slot 0: psum = 16*out
N_WARM = 15

f8t = ml_dtypes.float8_e4m3
bft = ml_dtypes.bfloat16


def build_kernel():
    nc = bacc.Bacc(
        "TRN2",
        target_bir_lowering=False,
        debug=False,
        num_devices=N_CORES,
        dynamic_dma_scratch_size=64,
    )
    xq8_d = nc.dram_tensor("xq8", [P, DT, SQ], FP8, kind="ExternalInput")
    xqb_d = nc.dram_tensor("xqb", [P, DT, P], BF16, kind="ExternalInput")
    m8_d = nc.dram_tensor("m8", [P, DT, DT, P], FP8, kind="ExternalInput")
    mb_d = nc.dram_tensor("mb", [P, DT, DT, P], BF16, kind="ExternalInput")
    xt8_d = nc.dram_tensor("xt8", [P, DT, S], FP8, kind="ExternalInput")
    xtb_d = nc.dram_tensor("xtb", [P, DT, KCH], BF16, kind="ExternalInput")
    xn8_d = nc.dram_tensor("xn8", [P, S // P, D], FP8, kind="ExternalInput")
    xnb_d = nc.dram_tensor("xnb", [P, 2, D], BF16, kind="ExternalInput")
    wv8_d = nc.dram_tensor("wv8", [P, DT, E], FP8, kind="ExternalInput")
    wvb_d = nc.dram_tensor("wvb", [P, DT, E], BF16, kind="ExternalInput")
    msk_d = nc.dram_tensor("masks", [P, NSLOT, KCH], BF16, kind="ExternalInput")
    out_d = nc.dram_tensor("out", [SQ, E], BF16, kind="ExternalOutput")

    with tile.TileContext(nc) as tc, ExitStack() as ctx:
        # persistent inputs + long-lived intermediates
        kqv = ctx.enter_context(tc.tile_pool(name="kqv", bufs=1, side="right"))
        xq8 = kqv.tile([P, DT, SQ], FP8, tag="xq8")
        xqb = kqv.tile([P, DT, P], BF16, tag="xqb")
        m8 = kqv.tile([P, DT, DT, P], FP8, tag="m8")
        mb = kqv.tile([P, DT, DT, P], BF16, tag="mb")
        xt8 = kqv.tile([P, DT, S], FP8, tag="xt8")
        xtb = kqv.tile([P, DT, KCH], BF16, tag="xtb")
        xn8 = kqv.tile([P, S // P, D], FP8, tag="xn8")
        xnb = kqv.tile([P, 2, D], BF16, tag="xnb")
        wv8 = kqv.tile([P, DT, E], FP8, tag="wv8")
        wvb = kqv.tile([P, DT, E], BF16, tag="wvb")
        msk = kqv.tile([P, NSLOT, KCH], BF16, tag="msk")
        qm8 = kqv.tile([P, DT, 7 * P], FP8, tag="qm8")     # 64*qM, slots 1-7
        qmb = kqv.tile([P, DT, P], BF16, tag="qmb")        # qM, slot 0
        wxt8 = kqv.tile([P, NSLOT, DT, P], FP8, tag="wxt8")  # 16*(wbar x)^T
        wxtb = kqv.tile([P, DT, P], BF16, tag="wxtb")        # slot 0
        stt = kqv.tile([P, NSLOT, 4], F32, tag="st")
        ln16 = kqv.tile([P, 1], F32, tag="ln16")
        nc.gpsimd.memset(ln16[:], LN16)
        ident = kqv.tile([P, P], BF16, tag="ident")
        from concourse.masks import make_identity
        make_identity(nc, ident[:])

        # One rotating PSUM pool for every phase: 2-bank tiles, 4 bufs.
        # Phase boundaries then only wait ~3 tiles back instead of a full
        # pool handoff.
        psp = ctx.enter_context(tc.tile_pool(name="psp", bufs=4, space="PSUM"))

        # ---------------- warmup + qM projection ----------------
        with tc.tile_pool(name="wupool", bufs=1) as wupool:
            warm = wupool.tile([P, 512], BF16, tag="warm")
            nc.gpsimd.memset(warm[:], 0.0)
            wps = psp.tile([P, 2, 512], F32, tag="ps", name="wps")
            for i in range(N_WARM):
                nc.tensor.matmul(
                    wps[:, i % 2, :], lhsT=warm[:, 0:P], rhs=warm[:],
                    start=True, stop=True,
                )

            # input streaming on one queue, ordered by first use; m8 split
            # per j_t pair so qM can start as pieces land
            nc.sync.dma_start(xq8[:, :, 0:512], xq8_d[:, :, 0:512])
            nc.sync.dma_start(m8[:, 0:2], m8_d[:, 0:2])
            nc.sync.dma_start(xq8[:, :, 512:SQ], xq8_d[:, :, 512:SQ])
            for jt2 in range(2, DT, 2):
                nc.sync.dma_start(m8[:, jt2 : jt2 + 2], m8_d[:, jt2 : jt2 + 2])
            nc.sync.dma_start(xqb[:], xqb_d[:])
            nc.sync.dma_start(mb[:], mb_d[:])
            nc.sync.dma_start(xt8[:], xt8_d[:])
            nc.sync.dma_start(msk[:], msk_d[:])
            nc.sync.dma_start(xtb[:], xtb_d[:])
            nc.sync.dma_start(xn8[:], xn8_d[:])
            nc.sync.dma_start(xnb[:], xnb_d[:])
            nc.sync.dma_start(wv8[:], wv8_d[:])
            nc.sync.dma_start(wvb[:], wvb_d[:])

            # fp8 qM for slots 1-7 (query cols 0:896), DR over d-pairs
            for j_t in range(DT):
                ps = psp.tile([P, 2, 512], F32, tag="ps", name="qmp")
                if j_t == 0:
                    # h-outer: the first half-column pass only needs the
                    # first xq8 DMA piece, starting the stream ~3us earlier
                    loops = [[(h, c) for dp in range(4)]
                             for h, c in ((0, (0, 512)), (1, (512, 384)))]
                    order = [(dp, h, c) for h, c in ((0, (0, 512)), (1, (512, 384)))
                             for dp in range(4)]
                else:
                    order = [(dp, h, c) for dp in range(4)
                             for h, c in ((0, (0, 512)), (1, (512, 384)))]
                for dp, h, (c0, cw) in order:
                    nc.tensor.matmul(
                        ps[:, h, 0:cw],
                        lhsT=m8[:, j_t, 2 * dp : 2 * dp + 2, :],
                        rhs=xq8[:, 2 * dp : 2 * dp + 2, c0 : c0 + cw],
                        start=(dp == 0),
                        stop=(dp == 3),
                        perf_mode=DR,
                    )
                nc.scalar.mul(qm8[:, j_t, 0:512], ps[:, 0, :], C_Q)
                nc.vector.tensor_scalar_mul(qm8[:, j_t, 512:896], ps[:, 1, 0:384], C_Q)

            # bf16 qM for slot 0 (query cols 896:1024): all 8 j_t packed
            # into one psum tile (4 per bank), two wide copies out
            p0 = psp.tile([P, 2, 512], F32, tag="ps", name="q0p")
            for j_t in range(DT):
                g, o = divmod(j_t, 4)
                for d in range(DT):
                    nc.tensor.matmul(
                        p0[:, g, o * P : (o + 1) * P],
                        lhsT=mb[:, j_t, d, :],
                        rhs=xqb[:, d, :],
                        start=(d == 0),
                        stop=(d == DT - 1),
                    )
            nc.vector.tensor_copy(
                qmb[:, 0:4, :].rearrange("p a b -> p (a b)"), p0[:, 0, :]
            )
            nc.vector.tensor_copy(
                qmb[:, 4:8, :].rearrange("p a b -> p (a b)"), p0[:, 1, :]
            )

        # ---------------- QK + softmax (exp from PSUM, no max) ----------------
        # Each slot's scores are produced in <=1024-key chunks (one 2-bank
        # tile each) with a separate exp per chunk, so banks recycle fast.
        wtspool = ctx.enter_context(tc.tile_pool(name="wts", bufs=NSLOT))
        wtbpool = ctx.enter_context(tc.tile_pool(name="wtb", bufs=3))
        wt8pool = ctx.enter_context(tc.tile_pool(name="wt8", bufs=7))
        wts_all = {}
        wtb_pre = {}
        wt_pre = {}

        def emit_wtb(j, wts):
            nkb = CJ[j] * KCH // P
            wtb = wtbpool.tile([P, 16, P], BF16, tag="wtb", name="wtb")
            nc.sync.dma_start(wtb[:, 0:nkb, :], wts[:, 0 : CJ[j] * KCH], transpose=True)
            wtb_pre[j] = wtb

        def emit_cast(j):
            nkb = CJ[j] * KCH // P
            if j == 0:
                wt_pre[j] = wtb_pre[j]
                return
            wt8 = wt8pool.tile([P, 16, P], FP8, tag="wt8", name="wt8")
            nc.vector.tensor_copy(wt8[:, 0:nkb, :], wtb_pre[j][:, 0:nkb, :])
            wt_pre[j] = wt8
        for si, j in enumerate(SLOT_ORDER):
            C = CJ[j]
            L = C * KCH
            wts = wtspool.tile([P, S], BF16, tag="w", name="wts")
            wts_all[j] = wts
            st = stt[:, j, :]
            scale = AS_BF if j == 0 else AS_F8
            chunks = []  # (key0, [group sizes <=512])
            k0 = 0
            while k0 < L:
                cl = min(1024, L - k0)
                gs = [512] * (cl // 512) + ([cl % 512] if cl % 512 else [])
                chunks.append((k0, gs))
                k0 += cl
            nacc = 0
            for ci, (k0, gs) in enumerate(chunks):
                sc = psp.tile([P, 2, 512], F32, tag="ps", name="sc")
                if j == 0:
                    for d in range(DT):
                        nc.tensor.matmul(
                            sc[:, 0, 0:KCH],
                            lhsT=qmb[:, d, :],
                            rhs=xtb[:, d, :],
                            start=(d == 0),
                            stop=(d == DT - 1),
                        )
                else:
                    q0 = QCOL[j]
                    for dp in range(4):
                        for gi, ksz in enumerate(gs):
                            nc.tensor.matmul(
                                sc[:, gi, 0:ksz],
                                lhsT=qm8[:, 2 * dp : 2 * dp + 2, q0 : q0 + P],
                                rhs=xt8[:, 2 * dp : 2 * dp + 2,
                                        k0 + gi * 512 : k0 + gi * 512 + ksz],
                                start=(dp == 0),
                                stop=(dp == 3),
                                perf_mode=DR,
                            )
                cl = sum(gs)
                if ci == len(chunks) - 1:  # causal edge mask, last 256 keys
                    eg, eo = (cl - KCH) // 512, (cl - KCH) % 512
                    nc.vector.tensor_add(
                        sc[:, eg, eo : eo + KCH], sc[:, eg, eo : eo + KCH],
                        msk[:, j, :],
                    )
                nc.scalar.activation(
                    wts[:, k0 : k0 + cl],
                    sc[:].rearrange("p a b -> p (a b)")[:, 0:cl],
                    EXP, bias=ln16[:], scale=scale, accum_out=st[:, nacc : nacc + 1],
                )
                nacc += 1
            if nacc == 1:
                nc.vector.reciprocal(st[:, 3:4], st[:, 0:1])
            else:
                nc.vector.tensor_add(st[:, 2:3], st[:, 0:1], st[:, 1:2])
                nc.vector.reciprocal(st[:, 3:4], st[:, 2:3])


        # ---------------- pass A: w @ x  (transposes on the DMA xbar) --------
        with (
            tc.tile_pool(name="wxsb", bufs=3) as wxpool,
            tc.tile_pool(name="wxtb2", bufs=2) as wxtbpool,
        ):
            # All weight transposes run on the PE (it would otherwise idle
            # while the input DMA stream drains the transpose queue); pair
            # copies (bf16 psum -> fp8 sbuf) replace the cast.
            def emit_pe_wt(j, oi):
                C = CJ[j]
                dt_out = BF16 if j == 0 else FP8
                wt8 = wt8pool.tile([P, 16, P], dt_out,
                                   tag="wt8" + ("b" if j == 0 else ""),
                                   name="wt8", bufs=1 if j == 0 else 7)
                wts = wts_all[j]
                pr = 0
                while pr < C:
                    n = min(4, C - pr)
                    trp = psp.tile([P, 2, 512], F32, tag="ps", name="trp")
                    trv = trp[:].rearrange("p a b -> p (a b)").bitcast(BF16)
                    for q in range(n):
                        for s in range(2):
                            kb = 2 * (pr + q) + s
                            nc.tensor.transpose(
                                trv[:, (2 * q + s) * P : (2 * q + s + 1) * P],
                                wts[:, kb * P : (kb + 1) * P],
                                ident[:],
                            )
                    for q in range(n):
                        cp = (nc.vector.tensor_copy if (pr + q) % 2 == oi
                              else nc.scalar.copy)
                        cp(
                            wt8[:, 2 * (pr + q) : 2 * (pr + q) + 2, :],
                            trv[:, 2 * q * P : (2 * q + 2) * P].rearrange(
                                "p (a b) -> p a b", a=2
                            ),
                        )
                    pr += n
                wt_pre[j] = wt8

            for oi, j in enumerate((1, 3, 5, 0, 7, 6, 4, 2)):
                emit_pe_wt(j, oi)
                C = CJ[j]
                L = C * KCH
                nkb = L // P
                wts = wts_all[j]
                st = stt[:, j, :]
                po = psp.tile([P, 2, 512], F32, tag="ps", name="wxp")
                if j == 0:
                    wtb = wt_pre[j]
                    for kb in range(nkb):
                        for h in range(2):
                            nc.tensor.matmul(
                                po[:, h, :],
                                lhsT=wtb[:, kb, :],
                                rhs=xnb[:, kb, h * 512 : (h + 1) * 512],
                                start=(kb == 0),
                                stop=(kb == nkb - 1),
                            )
                    # st3b = 16/accum for the true-scale bf16 path
                    nc.scalar.mul(st[:, 0:1], st[:, 3:4], 16.0)
                    sc_ap = st[:, 0:1]
                else:
                    wt8 = wt_pre[j]
                    for i in range(C):
                        for h in range(2):
                            nc.tensor.matmul(
                                po[:, h, :],
                                lhsT=wt8[:, 2 * i : 2 * i + 2, :],
                                rhs=xn8[:, 2 * i : 2 * i + 2, h * 512 : (h + 1) * 512],
                                start=(i == 0),
                                stop=(i == C - 1),
                                perf_mode=DR,
                            )
                    sc_ap = st[:, 3:4]
                wx = wxpool.tile([P, E], BF16, tag="wx", name="wx_sb")
                nc.scalar.mul(wx[:, 0:512], po[:, 0, :], sc_ap)
                nc.vector.tensor_scalar_mul(wx[:, 512:1024], po[:, 1, :], sc_ap)
                if j == 0:
                    nc.sync.dma_start(wxtb[:], wx[:], transpose=True)
                else:
                    wxtb2 = wxtbpool.tile([P, DT, P], BF16, tag="wxtb", name="wxtb2")
                    nc.sync.dma_start(wxtb2[:], wx[:], transpose=True)
                    if si % 2 == 0 or si >= 5:
                        nc.vector.tensor_copy(wxt8[:, j], wxtb2[:])
                    else:
                        nc.scalar.copy(wxt8[:, j], wxtb2[:])

        # ---------------- pass B: (w x) @ Wv^T, DMA out ----------------
        if True:
            for j in SLOT_ORDER:
                po = psp.tile([P, 2, 512], F32, tag="ps", name="pvp")
                if j == 0:
                    for d in range(DT):
                        for h in range(2):
                            nc.tensor.matmul(
                                po[:, h, :],
                                lhsT=wxtb[:, d, :],
                                rhs=wvb[:, d, h * 512 : (h + 1) * 512],
                                start=(d == 0),
                                stop=(d == DT - 1),
                            )
                    oscale = OUT_BF
                else:
                    for dp in range(4):
                        for h in range(2):
                            nc.tensor.matmul(
                                po[:, h, :],
                                lhsT=wxt8[:, j, 2 * dp : 2 * dp + 2, :],
                                rhs=wv8[:, 2 * dp : 2 * dp + 2, h * 512 : (h + 1) * 512],
                                start=(dp == 0),
                                stop=(dp == 3),
                                perf_mode=DR,
                            )
                    oscale = OUT_F8
                ot = wtspool.tile([P, E], BF16, tag="ot", name="ot", bufs=4)
                if j == SLOT_ORDER[-1]:
                    nc.scalar.mul(ot[:, 0:256], po[:, 0, 0:256], oscale)
                    nc.vector.tensor_scalar_mul(ot[:, 256:512], po[:, 0, 256:512], oscale)
                    nc.scalar.mul(ot[:, 512:768], po[:, 1, 0:256], oscale)
                    nc.vector.tensor_scalar_mul(ot[:, 768:1024], po[:, 1, 256:512], oscale)
                else:
                    nc.scalar.mul(ot[:, 0:512], po[:, 0, :], oscale)
                    nc.vector.tensor_scalar_mul(ot[:, 512:1024], po[:, 1, :], oscale)
                q0 = QCOL[j]
                nc.sync.dma_start(out_d[q0 : q0 + P, :], ot[:])

    nc.compile()
    return nc


_NC_CACHE = None


def _get_nc():
    global _NC_CACHE
    if _NC_CACHE is None:
        _NC_CACHE = build_kernel()
    return _NC_CACHE


def _q8(a, scale):
    return np.clip(a * scale, -240.0, 240.0).astype(f8t)


def _pack_inputs(x, Wq, Wk, Wv):
    """Host-side relayout, weight folding, and quantization."""
    M = (Wq.T.astype(np.float64) @ Wk.astype(np.float64)).astype(np.float32)
    # packed as lhsT slices: [p, j_t, d, j_local] = M[d*128+p, j_t*128+j_local]
    mp = M.reshape(DT, P, DT, P).transpose(1, 2, 0, 3)
    m8 = np.ascontiguousarray(_q8(mp, S_M))
    mb = np.ascontiguousarray(mp.astype(bft))
    # Wv^T packed d-outer: [p, d, e] = Wv[e, d*128+p]
    wvp = Wv.reshape(E, DT, P).transpose(2, 1, 0)
    wv8 = np.ascontiguousarray(_q8(wvp, S_V))
    wvb = np.ascontiguousarray(wvp.astype(bft))

    def packmask(blocks):
        m = np.zeros((NSLOT, P, KCH), np.float32)
        for j, blk in enumerate(blocks):
            cc = np.arange(KCH)[None, :] + (CJ[j] - 1) * KCH
            rr = np.arange(P)[:, None] + blk * P
            m[j] = np.where(cc <= rr, 0.0, MASK_VAL)
        return np.ascontiguousarray(m.transpose(1, 0, 2).astype(bft))

    masks = [packmask(QBLOCKS[0]), packmask(QBLOCKS[1])]

    in_maps = []
    for c in range(N_CORES):
        b, h = divmod(c, 2)
        xb = x[b]  # [S, D]
        xt = xb.reshape(S, DT, P).transpose(2, 1, 0)       # [p, d, s]
        xnat = xb.reshape(S // P, P, D).transpose(1, 0, 2)  # [p, kb, d]
        # query rows: slots 1..7 first, slot 0 last
        rows = np.concatenate(
            [np.arange(QBLOCKS[h][j] * P, (QBLOCKS[h][j] + 1) * P)
             for j in list(range(1, NSLOT)) + [0]]
        )
        xqt = xb[rows].reshape(SQ, DT, P).transpose(2, 1, 0)  # [p, d, q]
        in_maps.append(
            {
                "xq8": np.ascontiguousarray(_q8(xqt, S_X)),
                "xqb": np.ascontiguousarray(xqt[:, :, 7 * P :].astype(bft)),
                "m8": m8,
                "mb": mb,
                "xt8": np.ascontiguousarray(_q8(xt, S_X)),
                "xtb": np.ascontiguousarray(xt[:, :, 0:KCH].astype(bft)),
                "xn8": np.ascontiguousarray(_q8(xnat, S_X)),
                "xnb": np.ascontiguousarray(xnat[:, 0:2, :].astype(bft)),
                "wv8": wv8,
                "wvb": wvb,
                "masks": masks[h],
            }
        )
    return in_maps


def kernel(x, Wq, Wk, Wv, _spmd_kwargs=None, _results_out=None):
    x = np.asarray(x, dtype=np.float32)
    Wq = np.asarray(Wq, dtype=np.float32)
    Wk = np.asarray(Wk, dtype=np.float32)
    Wv = np.asarray(Wv, dtype=np.float32)
    assert x.shape == (B, S, D)

    nc = _get_nc()
    in_maps = _pack_inputs(x, Wq, Wk, Wv)
    res = run_bass_kernel_spmd(
        nc, in_maps, list(range(N_CORES)), **(_spmd_kwargs or {})
    )
    if _results_out is not None:
        _results_out.append(res)

    out = np.empty((B, S, E), np.float32)
    for c in range(N_CORES):
        b, h = divmod(c, 2)
        o = res.results[c]["out"].astype(np.float32)
        for j in range(NSLOT):
            blk = QBLOCKS[h][j]
            q0 = QCOL[j]
            out[b, blk * P : (blk + 1) * P, :] = o[q0 : q0 + P, :]
    return out


# revision 38
# speedup vs baseline: 1.0041x; 1.0041x over previous
"""Trainium2 Bass kernel for single-head causal attention.

Problem: x[4,2048,1024] f32; Wq/Wk/Wv [1024,1024] (torch Linear layout, y = x@W.T).
  q,k,v = x@W.T ; scores = q@k.T (causal masked, scaled 1/sqrt(1024)) ;
  out = softmax(scores)@v.

Algebra: with M := Wq^T Wk folded on the host, scores = xq M xk^T (the K
projection disappears; x^T itself is the key matrix), and
out = softmax(.) @ x @ Wv^T (the V projection becomes a postmultiply).

Sharding: 2 cores per batch, zig-zag query-block split (core h=0 gets blocks
[0,15,2,13,4,11,6,9], h=1 [1,14,3,12,5,10,7,8]) so all 8 cores run one SPMD
program with identical causal extents CJ=[1,8,2,7,3,6,4,5] (x 256 keys).

Precision: slots 1..7 (blocks 2..15, causal extent >= 512) run the entire
matmul chain in fp8-e4m3 with DoubleRow perf mode (2 contraction tiles per
instruction, ~1.8x PE throughput); slot 0 (blocks 0/1, peaked attention where
fp8 noise concentrates) stays bf16. Verified error ~0.7% vs the 2% gate.
Scale bookkeeping (powers of 2 throughout): x*16, M*512, Wv^T*2048 on host;
qMT8 = qM_psum/128; QK psum = 1024*scores -> exp scale 2^-15 with bias ln16
so wts = 16*exp(s/32); pass-A psum * (1/accum) = 16*normalized(w)@x; pass-B
psum * 2^-15 = out. No max-subtraction (|scores|/32 < 2, exp cannot overflow).

Structure per core (single rotating 2-bank PSUM pool spans all phases so
phase boundaries never wait on a full pool handoff):
  warmup MMs (HAM un-throttle, sized to the input-DMA arrival) ->
  qM: fp8-DR for query cols 0:896 (slots 1-7) + bf16 for cols 896:1024
      (slot 0); queries are column-reordered so the fp8 slots are contiguous ->
  QK per slot (big/small order, slot 0 mid): dp-outer DR MMs into <=1024-key
      PSUM chunks, causal edge mask added in-PSUM by DVE, exp straight from
      PSUM per chunk (ACT, accum_out = row sum), reciprocal on DVE ->
  pass A: ALL weight transposes run on the PE itself, grouped ahead of the
      w@x matmuls (the PE would otherwise idle while the input-DMA queue
      drains, and a grouped block avoids interleaving transpose-mode
      LDWEIGHTS with in-flight DR matmuls, which corrupted weights when
      tried inline); bf16-psum pair copies emit fp8 directly. DR MMs w @ x
      accumulate over key blocks; the PSUM->SBUF copy applies 1/sum ->
  wx_sb --DMA-transpose--> wxT -> fp8 (DVE/ACT alternating) ->
  pass B: DR MMs (16*wbar x) @ (2048*Wv^T), scaled bf16 out, DMA out.
Queues: all inputs stream on the sync HWDGE queue in first-use order; all
xbar transposes also ride the sync queue (two queues would race the xbar);
output DMAs fill the sync queue tail.
"""

from contextlib import ExitStack

import ml_dtypes
import numpy as np

import concourse.mybir as mybir
import concourse.tile as tile
from concourse import bacc
from concourse.bass_utils import run_bass_kernel_spmd

B, S, D, E = 4, 2048, 1024, 1024
P = 128
N_CORES = 8
DT = D // P          # 8 contraction tiles
SQ = S // 2          # 1024 query rows per core
KCH = 256            # causal-length granularity
NSLOT = SQ // P      # 8 query slots per core

QBLOCKS = [[0, 15, 2, 13, 4, 11, 6, 9], [1, 14, 3, 12, 5, 10, 7, 8]]
CJ = [(b + 2) // 2 for b in QBLOCKS[0]]       # [1,8,2,7,3,6,4,5]
assert CJ == [(b + 2) // 2 for b in QBLOCKS[1]]
# fp8 slots 1..7 at query columns (j-1)*128; bf16 slot 0 at columns 896:1024
QCOL = [7 * P] + [(j - 1) * P for j in range(1, NSLOT)]
SLOT_ORDER = [1, 3, 5, 0, 7, 6, 4, 2]  # big first; bf16 slot 0 mid; small last

F32 = mybir.dt.float32
BF16 = mybir.dt.bfloat16
FP8 = mybir.dt.float8e4
DR = mybir.MatmulPerfMode.DoubleRow
EXP = mybir.ActivationFunctionType.Exp
LN16 = float(np.log(16.0))
MASK_VAL = -1.0e9

S_X = 16.0           # fp8 scale for x-derived tensors
S_M = 512.0          # fp8 scale for M
S_V = 2048.0         # fp8 scale for Wv^T
C_Q = 1.0 / 128.0    # qM psum -> qMT8 copy scale (qMT8 = 64*qM_true)
AS_F8 = 1.0 / 32768.0   # QK psum scale for fp8 slots: (1/32) / 1024
AS_BF = 1.0 / 32.0      # slot-0 QK psum is true-scale
OUT_F8 = 1.0 / 32768.0  # pass-B psum scale, fp8 slots: 1/(16*2048)
OUT_BF = 1.0 / 16.0     # slot 0: psum = 16*out
N_WARM = 22

f8t = ml_dtypes.float8_e4m3
bft = ml_dtypes.bfloat16


def build_kernel():
    nc = bacc.Bacc(
        "TRN2",
        target_bir_lowering=False,
        debug=False,
        num_devices=N_CORES,
        dynamic_dma_scratch_size=64,
    )
    xq8_d = nc.dram_tensor("xq8", [P, DT, SQ], FP8, kind="ExternalInput")
    xqb_d = nc.dram_tensor("xqb", [P, DT, P], BF16, kind="ExternalInput")
    m8_d = nc.dram_tensor("m8", [P, DT, DT, P], FP8, kind="ExternalInput")
    mb_d = nc.dram_tensor("mb", [P, DT, DT, P], BF16, kind="ExternalInput")
    xt8_d = nc.dram_tensor("xt8", [P, DT, S], FP8, kind="ExternalInput")
    xtb_d = nc.dram_tensor("xtb", [P, DT, KCH], BF16, kind="ExternalInput")
    xn8_d = nc.dram_tensor("xn8", [P, S // P, D], FP8, kind="ExternalInput")
    xnb_d = nc.dram_tensor("xnb", [P, 2, D], BF16, kind="ExternalInput")
    wv8_d = nc.dram_tensor("wv8", [P, DT, E], FP8, kind="ExternalInput")
    wvb_d = nc.dram_tensor("wvb", [P, DT, E], BF16, kind="ExternalInput")
    msk_d = nc.dram_tensor("masks", [P, NSLOT, KCH], BF16, kind="ExternalInput")
    out_d = nc.dram_tensor("out", [SQ, E], BF16, kind="ExternalOutput")

    with tile.TileContext(nc) as tc, ExitStack() as ctx:
        # persistent inputs + long-lived intermediates
        kqv = ctx.enter_context(tc.tile_pool(name="kqv", bufs=1, side="right"))
        xq8 = kqv.tile([P, DT, SQ], FP8, tag="xq8")
        xqb = kqv.tile([P, DT, P], BF16, tag="xqb")
        m8 = kqv.tile([P, DT, DT, P], FP8, tag="m8")
        mb = kqv.tile([P, DT, DT, P], BF16, tag="mb")
        xt8 = kqv.tile([P, DT, S], FP8, tag="xt8")
        xtb = kqv.tile([P, DT, KCH], BF16, tag="xtb")
        xn8 = kqv.tile([P, S // P, D], FP8, tag="xn8")
        xnb = kqv.tile([P, 2, D], BF16, tag="xnb")
        wv8 = kqv.tile([P, DT, E], FP8, tag="wv8")
        wvb = kqv.tile([P, DT, E], BF16, tag="wvb")
        msk = kqv.tile([P, NSLOT, KCH], BF16, tag="msk")
        qm8 = kqv.tile([P, DT, 7 * P], FP8, tag="qm8")     # 64*qM, slots 1-7
        qmb = kqv.tile([P, DT, P], BF16, tag="qmb")        # qM, slot 0
        wxt8 = kqv.tile([P, NSLOT, DT, P], FP8, tag="wxt8")  # 16*(wbar x)^T
        wxtb = kqv.tile([P, DT, P], BF16, tag="wxtb")        # slot 0
        stt = kqv.tile([P, NSLOT, 4], F32, tag="st")
        ln16 = kqv.tile([P, 1], F32, tag="ln16")
        nc.gpsimd.memset(ln16[:], LN16)
        ident = kqv.tile([P, P], BF16, tag="ident")
        from concourse.masks import make_identity
        make_identity(nc, ident[:])

        # One rotating PSUM pool for every phase: 2-bank tiles, 4 bufs.
        # Phase boundaries then only wait ~3 tiles back instead of a full
        # pool handoff.
        psp = ctx.enter_context(tc.tile_pool(name="psp", bufs=4, space="PSUM"))

        # ---------------- warmup + qM projection ----------------
        with tc.tile_pool(name="wupool", bufs=1) as wupool:
            warm = wupool.tile([P, 512], BF16, tag="warm")
            nc.gpsimd.memset(warm[:], 0.0)
            wps = psp.tile([P, 2, 512], F32, tag="ps", name="wps")
            for i in range(N_WARM):
                nc.tensor.matmul(
                    wps[:, i % 2, :], lhsT=warm[:, 0:P], rhs=warm[:],
                    start=True, stop=True,
                )

            # input streaming on one queue, ordered by first use; m8 split
            # per j_t pair so qM can start as pieces land
            nc.sync.dma_start(xq8[:], xq8_d[:])
            for jt2 in range(0, DT, 2):
                nc.sync.dma_start(m8[:, jt2 : jt2 + 2], m8_d[:, jt2 : jt2 + 2])
            nc.sync.dma_start(xqb[:], xqb_d[:])
            nc.sync.dma_start(mb[:], mb_d[:])
            nc.sync.dma_start(xt8[:], xt8_d[:])
            nc.sync.dma_start(msk[:], msk_d[:])
            nc.sync.dma_start(xtb[:], xtb_d[:])
            nc.sync.dma_start(xn8[:], xn8_d[:])
            nc.sync.dma_start(xnb[:], xnb_d[:])
            nc.sync.dma_start(wv8[:], wv8_d[:])
            nc.sync.dma_start(wvb[:], wvb_d[:])

            # fp8 qM for slots 1-7 (query cols 0:896), DR over d-pairs
            for j_t in range(DT):
                ps = psp.tile([P, 2, 512], F32, tag="ps", name="qmp")
                for dp, h, (c0, cw) in [(dp, h, c) for dp in range(4)
                                        for h, c in ((0, (0, 512)), (1, (512, 384)))]:
                    nc.tensor.matmul(
                        ps[:, h, 0:cw],
                        lhsT=m8[:, j_t, 2 * dp : 2 * dp + 2, :],
                        rhs=xq8[:, 2 * dp : 2 * dp + 2, c0 : c0 + cw],
                        start=(dp == 0),
                        stop=(dp == 3),
                        perf_mode=DR,
                    )
                nc.scalar.mul(qm8[:, j_t, 0:512], ps[:, 0, :], C_Q)
                nc.vector.tensor_scalar_mul(qm8[:, j_t, 512:896], ps[:, 1, 0:384], C_Q)

            # bf16 qM for slot 0 (query cols 896:1024): all 8 j_t packed
            # into one psum tile (4 per bank), two wide copies out
            p0 = psp.tile([P, 2, 512], F32, tag="ps", name="q0p")
            for j_t in range(DT):
                g, o = divmod(j_t, 4)
                for d in range(DT):
                    nc.tensor.matmul(
                        p0[:, g, o * P : (o + 1) * P],
                        lhsT=mb[:, j_t, d, :],
                        rhs=xqb[:, d, :],
                        start=(d == 0),
                        stop=(d == DT - 1),
                    )
            nc.vector.tensor_copy(
                qmb[:, 0:4, :].rearrange("p a b -> p (a b)"), p0[:, 0, :]
            )
            nc.vector.tensor_copy(
                qmb[:, 4:8, :].rearrange("p a b -> p (a b)"), p0[:, 1, :]
            )

        # ---------------- QK + softmax (exp from PSUM, no max) ----------------
        # Each slot's scores are produced in <=1024-key chunks (one 2-bank
        # tile each) with a separate exp per chunk, so banks recycle fast.
        wtspool = ctx.enter_context(tc.tile_pool(name="wts", bufs=NSLOT))
        wtbpool = ctx.enter_context(tc.tile_pool(name="wtb", bufs=3))
        wt8pool = ctx.enter_context(tc.tile_pool(name="wt8", bufs=7))
        wts_all = {}
        wtb_pre = {}
        wt_pre = {}

        def emit_wtb(j, wts):
            nkb = CJ[j] * KCH // P
            wtb = wtbpool.tile([P, 16, P], BF16, tag="wtb", name="wtb")
            nc.sync.dma_start(wtb[:, 0:nkb, :], wts[:, 0 : CJ[j] * KCH], transpose=True)
            wtb_pre[j] = wtb

        def emit_cast(j):
            nkb = CJ[j] * KCH // P
            if j == 0:
                wt_pre[j] = wtb_pre[j]
                return
            wt8 = wt8pool.tile([P, 16, P], FP8, tag="wt8", name="wt8")
            nc.vector.tensor_copy(wt8[:, 0:nkb, :], wtb_pre[j][:, 0:nkb, :])
            wt_pre[j] = wt8
        for si, j in enumerate(SLOT_ORDER):
            C = CJ[j]
            L = C * KCH
            wts = wtspool.tile([P, S], BF16, tag="w", name="wts")
            wts_all[j] = wts
            st = stt[:, j, :]
            scale = AS_BF if j == 0 else AS_F8
            chunks = []  # (key0, [group sizes <=512])
            k0 = 0
            while k0 < L:
                cl = min(1024, L - k0)
                gs = [512] * (cl // 512) + ([cl % 512] if cl % 512 else [])
                chunks.append((k0, gs))
                k0 += cl
            nacc = 0
            for ci, (k0, gs) in enumerate(chunks):
                sc = psp.tile([P, 2, 512], F32, tag="ps", name="sc")
                if j == 0:
                    for d in range(DT):
                        nc.tensor.matmul(
                            sc[:, 0, 0:KCH],
                            lhsT=qmb[:, d, :],
                            rhs=xtb[:, d, :],
                            start=(d == 0),
                            stop=(d == DT - 1),
                        )
                else:
                    q0 = QCOL[j]
                    for dp in range(4):
                        for gi, ksz in enumerate(gs):
                            nc.tensor.matmul(
                                sc[:, gi, 0:ksz],
                                lhsT=qm8[:, 2 * dp : 2 * dp + 2, q0 : q0 + P],
                                rhs=xt8[:, 2 * dp : 2 * dp + 2,
                                        k0 + gi * 512 : k0 + gi * 512 + ksz],
                                start=(dp == 0),
                                stop=(dp == 3),
                                perf_mode=DR,
                            )
                cl = sum(gs)
                if ci == len(chunks) - 1:  # causal edge mask, last 256 keys
                    eg, eo = (cl - KCH) // 512, (cl - KCH) % 512
                    nc.vector.tensor_add(
                        sc[:, eg, eo : eo + KCH], sc[:, eg, eo : eo + KCH],
                        msk[:, j, :],
                    )
                nc.scalar.activation(
                    wts[:, k0 : k0 + cl],
                    sc[:].rearrange("p a b -> p (a b)")[:, 0:cl],
                    EXP, bias=ln16[:], scale=scale, accum_out=st[:, nacc : nacc + 1],
                )
                nacc += 1
            if nacc == 1:
                nc.vector.reciprocal(st[:, 3:4], st[:, 0:1])
            else:
                nc.vector.tensor_add(st[:, 2:3], st[:, 0:1], st[:, 1:2])
                nc.vector.reciprocal(st[:, 3:4], st[:, 2:3])


        # ---------------- pass A: w @ x  (transposes on the DMA xbar) --------
        with (
            tc.tile_pool(name="wxsb", bufs=3) as wxpool,
            tc.tile_pool(name="wxtb2", bufs=2) as wxtbpool,
        ):
            # All weight transposes run on the PE (it would otherwise idle
            # while the input DMA stream drains the transpose queue); pair
            # copies (bf16 psum -> fp8 sbuf) replace the cast.
            def emit_pe_wt(j, oi):
                C = CJ[j]
                dt_out = BF16 if j == 0 else FP8
                wt8 = wt8pool.tile([P, 16, P], dt_out,
                                   tag="wt8" + ("b" if j == 0 else ""),
                                   name="wt8", bufs=1 if j == 0 else 7)
                wts = wts_all[j]
                pr = 0
                while pr < C:
                    n = min(4, C - pr)
                    trp = psp.tile([P, 2, 512], F32, tag="ps", name="trp")
                    trv = trp[:].rearrange("p a b -> p (a b)").bitcast(BF16)
                    for q in range(n):
                        for s in range(2):
                            kb = 2 * (pr + q) + s
                            nc.tensor.transpose(
                                trv[:, (2 * q + s) * P : (2 * q + s + 1) * P],
                                wts[:, kb * P : (kb + 1) * P],
                                ident[:],
                            )
                    for q in range(n):
                        cp = (nc.vector.tensor_copy if (pr + q) % 2 == oi
                              else nc.scalar.copy)
                        cp(
                            wt8[:, 2 * (pr + q) : 2 * (pr + q) + 2, :],
                            trv[:, 2 * q * P : (2 * q + 2) * P].rearrange(
                                "p (a b) -> p a b", a=2
                            ),
                        )
                    pr += n
                wt_pre[j] = wt8

            for oi, j in enumerate((1, 3, 5, 0, 7, 6, 4, 2)):
                emit_pe_wt(j, oi)
                C = CJ[j]
                L = C * KCH
                nkb = L // P
                wts = wts_all[j]
                st = stt[:, j, :]
                po = psp.tile([P, 2, 512], F32, tag="ps", name="wxp")
                if j == 0:
                    wtb = wt_pre[j]
                    for kb in range(nkb):
                        for h in range(2):
                            nc.tensor.matmul(
                                po[:, h, :],
                                lhsT=wtb[:, kb, :],
                                rhs=xnb[:, kb, h * 512 : (h + 1) * 512],
                                start=(kb == 0),
                                stop=(kb == nkb - 1),
                            )
                    # st3b = 16/accum for the true-scale bf16 path
                    nc.scalar.mul(st[:, 0:1], st[:, 3:4], 16.0)
                    sc_ap = st[:, 0:1]
                else:
                    wt8 = wt_pre[j]
                    for i in range(C):
                        for h in range(2):
                            nc.tensor.matmul(
                                po[:, h, :],
                                lhsT=wt8[:, 2 * i : 2 * i + 2, :],
                                rhs=xn8[:, 2 * i : 2 * i + 2, h * 512 : (h + 1) * 512],
                                start=(i == 0),
                                stop=(i == C - 1),
                                perf_mode=DR,
                            )
                    sc_ap = st[:, 3:4]
                wx = wxpool.tile([P, E], BF16, tag="wx", name="wx_sb")
                nc.scalar.mul(wx[:, 0:512], po[:, 0, :], sc_ap)
                nc.vector.tensor_scalar_mul(wx[:, 512:1024], po[:, 1, :], sc_ap)
                if j == 0:
                    nc.sync.dma_start(wxtb[:], wx[:], transpose=True)
                else:
                    wxtb2 = wxtbpool.tile([P, DT, P], BF16, tag="wxtb", name="wxtb2")
                    nc.sync.dma_start(wxtb2[:], wx[:], transpose=True)
                    if si % 2 == 0 or si >= 5:
                        nc.vector.tensor_copy(wxt8[:, j], wxtb2[:])
                    else:
                        nc.scalar.copy(wxt8[:, j], wxtb2[:])

        # ---------------- pass B: (w x) @ Wv^T, DMA out ----------------
        if True:
            for j in SLOT_ORDER:
                po = psp.tile([P, 2, 512], F32, tag="ps", name="pvp")
                if j == 0:
                    for d in range(DT):
                        for h in range(2):
                            nc.tensor.matmul(
                                po[:, h, :],
                                lhsT=wxtb[:, d, :],
                                rhs=wvb[:, d, h * 512 : (h + 1) * 512],
                                start=(d == 0),
                                stop=(d == DT - 1),
                            )
                    oscale = OUT_BF
                else:
                    for dp in range(4):
                        for h in range(2):
                            nc.tensor.matmul(
                                po[:, h, :],
                                lhsT=wxt8[:, j, 2 * dp : 2 * dp + 2, :],
                                rhs=wv8[:, 2 * dp : 2 * dp + 2, h * 512 : (h + 1) * 512],
                                start=(dp == 0),
                                stop=(dp == 3),
                                perf_mode=DR,
                            )
                    oscale = OUT_F8
                ot = wtspool.tile([P, E], BF16, tag="ot", name="ot", bufs=4)
                if j == SLOT_ORDER[-1]:
                    nc.scalar.mul(ot[:, 0:256], po[:, 0, 0:256], oscale)
                    nc.vector.tensor_scalar_mul(ot[:, 256:512], po[:, 0, 256:512], oscale)
                    nc.scalar.mul(ot[:, 512:768], po[:, 1, 0:256], oscale)
                    nc.vector.tensor_scalar_mul(ot[:, 768:1024], po[:, 1, 256:512], oscale)
                else:
                    nc.scalar.mul(ot[:, 0:512], po[:, 0, :], oscale)
                    nc.vector.tensor_scalar_mul(ot[:, 512:1024], po[:, 1, :], oscale)
                q0 = QCOL[j]
                nc.sync.dma_start(out_d[q0 : q0 + P, :], ot[:])

    nc.compile()
    return nc


_NC_CACHE = None


def _get_nc():
    global _NC_CACHE
    if _NC_CACHE is None:
        _NC_CACHE = build_kernel()
    return _NC_CACHE


def _q8(a, scale):
    return np.clip(a * scale, -240.0, 240.0).astype(f8t)


def _pack_inputs(x, Wq, Wk, Wv):
    """Host-side relayout, weight folding, and quantization."""
    M = (Wq.T.astype(np.float64) @ Wk.astype(np.float64)).astype(np.float32)
    # packed as lhsT slices: [p, j_t, d, j_local] = M[d*128+p, j_t*128+j_local]
    mp = M.reshape(DT, P, DT, P).transpose(1, 2, 0, 3)
    m8 = np.ascontiguousarray(_q8(mp, S_M))
    mb = np.ascontiguousarray(mp.astype(bft))
    # Wv^T packed d-outer: [p, d, e] = Wv[e, d*128+p]
    wvp = Wv.reshape(E, DT, P).transpose(2, 1, 0)
    wv8 = np.ascontiguousarray(_q8(wvp, S_V))
    wvb = np.ascontiguousarray(wvp.astype(bft))

    def packmask(blocks):
        m = np.zeros((NSLOT, P, KCH), np.float32)
        for j, blk in enumerate(blocks):
            cc = np.arange(KCH)[None, :] + (CJ[j] - 1) * KCH
            rr = np.arange(P)[:, None] + blk * P
            m[j] = np.where(cc <= rr, 0.0, MASK_VAL)
        return np.ascontiguousarray(m.transpose(1, 0, 2).astype(bft))

    masks = [packmask(QBLOCKS[0]), packmask(QBLOCKS[1])]

    in_maps = []
    for c in range(N_CORES):
        b, h = divmod(c, 2)
        xb = x[b]  # [S, D]
        xt = xb.reshape(S, DT, P).transpose(2, 1, 0)       # [p, d, s]
        xnat = xb.reshape(S // P, P, D).transpose(1, 0, 2)  # [p, kb, d]
        # query rows: slots 1..7 first, slot 0 last
        rows = np.concatenate(
            [np.arange(QBLOCKS[h][j] * P, (QBLOCKS[h][j] + 1) * P)
             for j in list(range(1, NSLOT)) + [0]]
        )
        xqt = xb[rows].reshape(SQ, DT, P).transpose(2, 1, 0)  # [p, d, q]
        in_maps.append(
            {
                "xq8": np.ascontiguousarray(_q8(xqt, S_X)),
                "xqb": np.ascontiguousarray(xqt[:, :, 7 * P :].astype(bft)),
                "m8": m8,
                "mb": mb,
                "xt8": np.ascontiguousarray(_q8(xt, S_X)),
                "xtb": np.ascontiguousarray(xt[:, :, 0:KCH].astype(bft)),
                "xn8": np.ascontiguousarray(_q8(xnat, S_X)),
                "xnb": np.ascontiguousarray(xnat[:, 0:2, :].astype(bft)),
                "wv8": wv8,
                "wvb": wvb,
                "masks": masks[h],
            }
        )
    return in_maps


def kernel(x, Wq, Wk, Wv, _spmd_kwargs=None, _results_out=None):
    x = np.asarray(x, dtype=np.float32)
    Wq = np.asarray(Wq, dtype=np.float32)
    Wk = np.asarray(Wk, dtype=np.float32)
    Wv = np.asarray(Wv, dtype=np.float32)
    assert x.shape == (B, S, D)

    nc = _get_nc()
    in_maps = _pack_inputs(x, Wq, Wk, Wv)
    res = run_bass_kernel_spmd(
        nc, in_maps, list(range(N_CORES)), **(_spmd_kwargs or {})
    )
    if _results_out is not None:
        _results_out.append(res)

    out = np.empty((B, S, E), np.float32)
    for c in range(N_CORES):
        b, h = divmod(c, 2)
        o = res.results[c]["out"].astype(np.float32)
        for j in range(NSLOT):
            blk = QBLOCKS[h][j]
            q0 = QCOL[j]
            out[b, blk * P : (blk + 1) * P, :] = o[q0 : q0 + P, :]
    return out


# revision 39
# speedup vs baseline: 1.0291x; 1.0249x over previous
"""Trainium2 Bass kernel for single-head causal attention.

Problem: x[4,2048,1024] f32; Wq/Wk/Wv [1024,1024] (torch Linear layout, y = x@W.T).
  q,k,v = x@W.T ; scores = q@k.T (causal masked, scaled 1/sqrt(1024)) ;
  out = softmax(scores)@v.

Algebra: with M := Wq^T Wk folded on the host, scores = xq M xk^T (the K
projection disappears; x^T itself is the key matrix), and
out = softmax(.) @ x @ Wv^T (the V projection becomes a postmultiply).

Sharding: 2 cores per batch, zig-zag query-block split (core h=0 gets blocks
[0,15,2,13,4,11,6,9], h=1 [1,14,3,12,5,10,7,8]) so all 8 cores run one SPMD
program with identical causal extents CJ=[1,8,2,7,3,6,4,5] (x 256 keys).

Precision: slots 1..7 (blocks 2..15, causal extent >= 512) run the entire
matmul chain in fp8-e4m3 with DoubleRow perf mode (2 contraction tiles per
instruction, ~1.8x PE throughput); slot 0 (blocks 0/1, peaked attention where
fp8 noise concentrates) stays bf16. Verified error ~0.7% vs the 2% gate.
Scale bookkeeping (powers of 2 throughout): x*16, M*512, Wv^T*2048 on host;
qMT8 = qM_psum/128; QK psum = 1024*scores -> exp scale 2^-15 with bias ln16
so wts = 16*exp(s/32); pass-A psum * (1/accum) = 16*normalized(w)@x; pass-B
psum * 2^-15 = out. No max-subtraction (|scores|/32 < 2, exp cannot overflow).

Structure per core (single rotating 2-bank PSUM pool spans all phases so
phase boundaries never wait on a full pool handoff):
  warmup MMs (HAM un-throttle, sized to the input-DMA arrival) ->
  qM: fp8-DR for query cols 0:896 (slots 1-7) + bf16 for cols 896:1024
      (slot 0); queries are column-reordered so the fp8 slots are contiguous ->
  QK per slot (big/small order, slot 0 mid): dp-outer DR MMs into <=1024-key
      PSUM chunks, causal edge mask added in-PSUM by DVE, exp straight from
      PSUM per chunk (ACT, accum_out = row sum), reciprocal on DVE ->
  pass A: ALL weight transposes run on the PE itself, grouped ahead of the
      w@x matmuls (the PE would otherwise idle while the input-DMA queue
      drains, and a grouped block avoids interleaving transpose-mode
      LDWEIGHTS with in-flight DR matmuls, which corrupted weights when
      tried inline); bf16-psum pair copies emit fp8 directly. DR MMs w @ x
      accumulate over key blocks; the PSUM->SBUF copy applies 1/sum ->
  wx_sb --DMA-transpose--> wxT -> fp8 (DVE/ACT alternating) ->
  pass B: DR MMs (16*wbar x) @ (2048*Wv^T), scaled bf16 out, DMA out.
Queues: all inputs stream on the sync HWDGE queue in first-use order; all
xbar transposes also ride the sync queue (two queues would race the xbar);
output DMAs fill the sync queue tail.
"""

from contextlib import ExitStack

import ml_dtypes
import numpy as np

import concourse.mybir as mybir
import concourse.tile as tile
from concourse import bacc
from concourse.bass_utils import run_bass_kernel_spmd

B, S, D, E = 4, 2048, 1024, 1024
P = 128
N_CORES = 8
DT = D // P          # 8 contraction tiles
SQ = S // 2          # 1024 query rows per core
KCH = 256            # causal-length granularity
NSLOT = SQ // P      # 8 query slots per core

QBLOCKS = [[0, 15, 2, 13, 4, 11, 6, 9], [1, 14, 3, 12, 5, 10, 7, 8]]
CJ = [(b + 2) // 2 for b in QBLOCKS[0]]       # [1,8,2,7,3,6,4,5]
assert CJ == [(b + 2) // 2 for b in QBLOCKS[1]]
# fp8 slots 1..7 at query columns (j-1)*128; bf16 slot 0 at columns 896:1024
QCOL = [7 * P] + [(j - 1) * P for j in range(1, NSLOT)]
SLOT_ORDER = [1, 3, 5, 0, 7, 6, 4, 2]  # big first; bf16 slot 0 mid; small last

F32 = mybir.dt.float32
BF16 = mybir.dt.bfloat16
FP8 = mybir.dt.float8e4
DR = mybir.MatmulPerfMode.DoubleRow
EXP = mybir.ActivationFunctionType.Exp
LN16 = float(np.log(16.0))
MASK_VAL = -1.0e9

S_X = 16.0           # fp8 scale for x-derived tensors
S_M = 512.0          # fp8 scale for M
S_V = 2048.0         # fp8 scale for Wv^T
C_Q = 1.0 / 128.0    # qM psum -> qMT8 copy scale (qMT8 = 64*qM_true)
AS_F8 = 1.0 / 32768.0   # QK psum scale for fp8 slots: (1/32) / 1024
AS_BF = 1.0 / 32.0      # slot-0 QK psum is true-scale
OUT_F8 = 1.0 / 32768.0  # pass-B psum scale, fp8 slots: 1/(16*2048)
OUT_BF = 1.0 / 16.0     # slot 0: psum = 16*out
N_WARM = 22

f8t = ml_dtypes.float8_e4m3
bft = ml_dtypes.bfloat16


def build_kernel():
    nc = bacc.Bacc(
        "TRN2",
        target_bir_lowering=False,
        debug=False,
        num_devices=N_CORES,
        dynamic_dma_scratch_size=64,
    )
    xq8_d = nc.dram_tensor("xq8", [P, DT, SQ], FP8, kind="ExternalInput")
    xqb_d = nc.dram_tensor("xqb", [P, DT, P], BF16, kind="ExternalInput")
    m8_d = nc.dram_tensor("m8", [P, DT, DT, P], FP8, kind="ExternalInput")
    mb_d = nc.dram_tensor("mb", [P, DT, DT, P], BF16, kind="ExternalInput")
    xt8_d = nc.dram_tensor("xt8", [P, DT, S], FP8, kind="ExternalInput")
    xtb_d = nc.dram_tensor("xtb", [P, DT, KCH], BF16, kind="ExternalInput")
    xn8_d = nc.dram_tensor("xn8", [P, S // P, D], FP8, kind="ExternalInput")
    xnb_d = nc.dram_tensor("xnb", [P, 2, D], BF16, kind="ExternalInput")
    wv8_d = nc.dram_tensor("wv8", [P, DT, E], FP8, kind="ExternalInput")
    wvb_d = nc.dram_tensor("wvb", [P, DT, E], BF16, kind="ExternalInput")
    msk_d = nc.dram_tensor("masks", [P, NSLOT, KCH], BF16, kind="ExternalInput")
    out_d = nc.dram_tensor("out", [SQ, E], BF16, kind="ExternalOutput")

    with tile.TileContext(nc) as tc, ExitStack() as ctx:
        # persistent inputs + long-lived intermediates
        kqv = ctx.enter_context(tc.tile_pool(name="kqv", bufs=1, side="right"))
        xq8 = kqv.tile([P, DT, SQ], FP8, tag="xq8")
        xqb = kqv.tile([P, DT, P], BF16, tag="xqb")
        m8 = kqv.tile([P, DT, DT, P], FP8, tag="m8")
        mb = kqv.tile([P, DT, DT, P], BF16, tag="mb")
        xt8 = kqv.tile([P, DT, S], FP8, tag="xt8")
        xtb = kqv.tile([P, DT, KCH], BF16, tag="xtb")
        xn8 = kqv.tile([P, S // P, D], FP8, tag="xn8")
        xnb = kqv.tile([P, 2, D], BF16, tag="xnb")
        wv8 = kqv.tile([P, DT, E], FP8, tag="wv8")
        wvb = kqv.tile([P, DT, E], BF16, tag="wvb")
        msk = kqv.tile([P, NSLOT, KCH], BF16, tag="msk")
        qm8 = kqv.tile([P, DT, 7 * P], FP8, tag="qm8")     # 64*qM, slots 1-7
        qmb = kqv.tile([P, DT, P], BF16, tag="qmb")        # qM, slot 0
        wxt8 = kqv.tile([P, NSLOT, DT, P], FP8, tag="wxt8")  # 16*(wbar x)^T
        wxtb = kqv.tile([P, DT, P], BF16, tag="wxtb")        # slot 0
        stt = kqv.tile([P, NSLOT, 4], F32, tag="st")
        ln16 = kqv.tile([P, 1], F32, tag="ln16")
        nc.gpsimd.memset(ln16[:], LN16)
        ident = kqv.tile([P, P], BF16, tag="ident")
        from concourse.masks import make_identity
        make_identity(nc, ident[:])

        # One rotating PSUM pool for every phase: 2-bank tiles, 4 bufs.
        # Phase boundaries then only wait ~3 tiles back instead of a full
        # pool handoff.
        psp = ctx.enter_context(tc.tile_pool(name="psp", bufs=4, space="PSUM"))

        # ---------------- warmup + qM projection ----------------
        with tc.tile_pool(name="wupool", bufs=1) as wupool:
            warm = wupool.tile([P, 512], BF16, tag="warm")
            nc.gpsimd.memset(warm[:], 0.0)
            wps = psp.tile([P, 2, 512], F32, tag="ps", name="wps")
            for i in range(N_WARM):
                nc.tensor.matmul(
                    wps[:, i % 2, :], lhsT=warm[:, 0:P], rhs=warm[:],
                    start=True, stop=True,
                )

            # input streaming on one queue, ordered by first use; m8 split
            # per j_t pair so qM can start as pieces land
            nc.sync.dma_start(xq8[:], xq8_d[:])
            for jt2 in range(0, DT, 2):
                nc.sync.dma_start(m8[:, jt2 : jt2 + 2], m8_d[:, jt2 : jt2 + 2])
            nc.sync.dma_start(xqb[:], xqb_d[:])
            nc.sync.dma_start(mb[:], mb_d[:])
            nc.sync.dma_start(xt8[:], xt8_d[:])
            nc.sync.dma_start(msk[:], msk_d[:])
            nc.sync.dma_start(xtb[:], xtb_d[:])
            nc.sync.dma_start(xn8[:], xn8_d[:])
            nc.sync.dma_start(xnb[:], xnb_d[:])
            nc.sync.dma_start(wv8[:], wv8_d[:])
            nc.sync.dma_start(wvb[:], wvb_d[:])

            # fp8 qM for slots 1-7 (query cols 0:896), DR over d-pairs
            for j_t in range(DT):
                ps = psp.tile([P, 2, 512], F32, tag="ps", name="qmp")
                for dp, h, (c0, cw) in [(dp, h, c) for dp in range(4)
                                        for h, c in ((0, (0, 512)), (1, (512, 384)))]:
                    nc.tensor.matmul(
                        ps[:, h, 0:cw],
                        lhsT=m8[:, j_t, 2 * dp : 2 * dp + 2, :],
                        rhs=xq8[:, 2 * dp : 2 * dp + 2, c0 : c0 + cw],
                        start=(dp == 0),
                        stop=(dp == 3),
                        perf_mode=DR,
                    )
                nc.scalar.mul(qm8[:, j_t, 0:512], ps[:, 0, :], C_Q)
                if j_t >= 6:
                    # QK's first slot gates on the last qm8 copies; keep them
                    # on ACT so the lagging DVE queue can't stall the PE
                    nc.scalar.mul(qm8[:, j_t, 512:896], ps[:, 1, 0:384], C_Q)
                else:
                    nc.vector.tensor_scalar_mul(qm8[:, j_t, 512:896], ps[:, 1, 0:384], C_Q)

            # bf16 qM for slot 0 (query cols 896:1024): all 8 j_t packed
            # into one psum tile (4 per bank), two wide copies out
            p0 = psp.tile([P, 2, 512], F32, tag="ps", name="q0p")
            for j_t in range(DT):
                g, o = divmod(j_t, 4)
                for d in range(DT):
                    nc.tensor.matmul(
                        p0[:, g, o * P : (o + 1) * P],
                        lhsT=mb[:, j_t, d, :],
                        rhs=xqb[:, d, :],
                        start=(d == 0),
                        stop=(d == DT - 1),
                    )
            nc.vector.tensor_copy(
                qmb[:, 0:4, :].rearrange("p a b -> p (a b)"), p0[:, 0, :]
            )
            nc.vector.tensor_copy(
                qmb[:, 4:8, :].rearrange("p a b -> p (a b)"), p0[:, 1, :]
            )

        # ---------------- QK + softmax (exp from PSUM, no max) ----------------
        # Each slot's scores are produced in <=1024-key chunks (one 2-bank
        # tile each) with a separate exp per chunk, so banks recycle fast.
        wtspool = ctx.enter_context(tc.tile_pool(name="wts", bufs=NSLOT))
        wtbpool = ctx.enter_context(tc.tile_pool(name="wtb", bufs=3))
        wt8pool = ctx.enter_context(tc.tile_pool(name="wt8", bufs=7))
        wts_all = {}
        wtb_pre = {}
        wt_pre = {}

        def emit_wtb(j, wts):
            nkb = CJ[j] * KCH // P
            wtb = wtbpool.tile([P, 16, P], BF16, tag="wtb", name="wtb")
            nc.sync.dma_start(wtb[:, 0:nkb, :], wts[:, 0 : CJ[j] * KCH], transpose=True)
            wtb_pre[j] = wtb

        def emit_cast(j):
            nkb = CJ[j] * KCH // P
            if j == 0:
                wt_pre[j] = wtb_pre[j]
                return
            wt8 = wt8pool.tile([P, 16, P], FP8, tag="wt8", name="wt8")
            nc.vector.tensor_copy(wt8[:, 0:nkb, :], wtb_pre[j][:, 0:nkb, :])
            wt_pre[j] = wt8
        for si, j in enumerate(SLOT_ORDER):
            C = CJ[j]
            L = C * KCH
            wts = wtspool.tile([P, S], BF16, tag="w", name="wts")
            wts_all[j] = wts
            st = stt[:, j, :]
            scale = AS_BF if j == 0 else AS_F8
            chunks = []  # (key0, [group sizes <=512])
            k0 = 0
            while k0 < L:
                cl = min(1024, L - k0)
                gs = [512] * (cl // 512) + ([cl % 512] if cl % 512 else [])
                chunks.append((k0, gs))
                k0 += cl
            nacc = 0
            for ci, (k0, gs) in enumerate(chunks):
                sc = psp.tile([P, 2, 512], F32, tag="ps", name="sc")
                if j == 0:
                    for d in range(DT):
                        nc.tensor.matmul(
                            sc[:, 0, 0:KCH],
                            lhsT=qmb[:, d, :],
                            rhs=xtb[:, d, :],
                            start=(d == 0),
                            stop=(d == DT - 1),
                        )
                else:
                    q0 = QCOL[j]
                    for dp in range(4):
                        for gi, ksz in enumerate(gs):
                            nc.tensor.matmul(
                                sc[:, gi, 0:ksz],
                                lhsT=qm8[:, 2 * dp : 2 * dp + 2, q0 : q0 + P],
                                rhs=xt8[:, 2 * dp : 2 * dp + 2,
                                        k0 + gi * 512 : k0 + gi * 512 + ksz],
                                start=(dp == 0),
                                stop=(dp == 3),
                                perf_mode=DR,
                            )
                cl = sum(gs)
                if ci == len(chunks) - 1:  # causal edge mask, last 256 keys
                    eg, eo = (cl - KCH) // 512, (cl - KCH) % 512
                    nc.vector.tensor_add(
                        sc[:, eg, eo : eo + KCH], sc[:, eg, eo : eo + KCH],
                        msk[:, j, :],
                    )
                nc.scalar.activation(
                    wts[:, k0 : k0 + cl],
                    sc[:].rearrange("p a b -> p (a b)")[:, 0:cl],
                    EXP, bias=ln16[:], scale=scale, accum_out=st[:, nacc : nacc + 1],
                )
                nacc += 1
            if nacc == 1:
                nc.vector.reciprocal(st[:, 3:4], st[:, 0:1])
            else:
                nc.vector.tensor_add(st[:, 2:3], st[:, 0:1], st[:, 1:2])
                nc.vector.reciprocal(st[:, 3:4], st[:, 2:3])


        # ---------------- pass A: w @ x  (transposes on the DMA xbar) --------
        with (
            tc.tile_pool(name="wxsb", bufs=3) as wxpool,
            tc.tile_pool(name="wxtb2", bufs=2) as wxtbpool,
        ):
            # All weight transposes run on the PE (it would otherwise idle
            # while the input DMA stream drains the transpose queue); pair
            # copies (bf16 psum -> fp8 sbuf) replace the cast.
            def emit_pe_wt(j, oi):
                C = CJ[j]
                dt_out = BF16 if j == 0 else FP8
                wt8 = wt8pool.tile([P, 16, P], dt_out,
                                   tag="wt8" + ("b" if j == 0 else ""),
                                   name="wt8", bufs=1 if j == 0 else 7)
                wts = wts_all[j]
                pr = 0
                while pr < C:
                    n = min(4, C - pr)
                    trp = psp.tile([P, 2, 512], F32, tag="ps", name="trp")
                    trv = trp[:].rearrange("p a b -> p (a b)").bitcast(BF16)
                    for q in range(n):
                        for s in range(2):
                            kb = 2 * (pr + q) + s
                            nc.tensor.transpose(
                                trv[:, (2 * q + s) * P : (2 * q + s + 1) * P],
                                wts[:, kb * P : (kb + 1) * P],
                                ident[:],
                            )
                    for q in range(n):
                        cp = (nc.vector.tensor_copy if (pr + q) % 2 == oi
                              else nc.scalar.copy)
                        cp(
                            wt8[:, 2 * (pr + q) : 2 * (pr + q) + 2, :],
                            trv[:, 2 * q * P : (2 * q + 2) * P].rearrange(
                                "p (a b) -> p a b", a=2
                            ),
                        )
                    pr += n
                wt_pre[j] = wt8

            for oi, j in enumerate((1, 3, 5, 0, 7, 6, 4, 2)):
                emit_pe_wt(j, oi)
                C = CJ[j]
                L = C * KCH
                nkb = L // P
                wts = wts_all[j]
                st = stt[:, j, :]
                po = psp.tile([P, 2, 512], F32, tag="ps", name="wxp")
                if j == 0:
                    wtb = wt_pre[j]
                    for kb in range(nkb):
                        for h in range(2):
                            nc.tensor.matmul(
                                po[:, h, :],
                                lhsT=wtb[:, kb, :],
                                rhs=xnb[:, kb, h * 512 : (h + 1) * 512],
                                start=(kb == 0),
                                stop=(kb == nkb - 1),
                            )
                    # st3b = 16/accum for the true-scale bf16 path
                    nc.scalar.mul(st[:, 0:1], st[:, 3:4], 16.0)
                    sc_ap = st[:, 0:1]
                else:
                    wt8 = wt_pre[j]
                    for i in range(C):
                        for h in range(2):
                            nc.tensor.matmul(
                                po[:, h, :],
                                lhsT=wt8[:, 2 * i : 2 * i + 2, :],
                                rhs=xn8[:, 2 * i : 2 * i + 2, h * 512 : (h + 1) * 512],
                                start=(i == 0),
                                stop=(i == C - 1),
                                perf_mode=DR,
                            )
                    sc_ap = st[:, 3:4]
                wx = wxpool.tile([P, E], BF16, tag="wx", name="wx_sb")
                nc.scalar.mul(wx[:, 0:512], po[:, 0, :], sc_ap)
                nc.vector.tensor_scalar_mul(wx[:, 512:1024], po[:, 1, :], sc_ap)
                if j == 0:
                    nc.sync.dma_start(wxtb[:], wx[:], transpose=True)
                else:
                    wxtb2 = wxtbpool.tile([P, DT, P], BF16, tag="wxtb", name="wxtb2")
                    nc.sync.dma_start(wxtb2[:], wx[:], transpose=True)
                    if si % 2 == 0 or si >= 5:
                        nc.vector.tensor_copy(wxt8[:, j], wxtb2[:])
                    else:
                        nc.scalar.copy(wxt8[:, j], wxtb2[:])

        # ---------------- pass B: (w x) @ Wv^T, DMA out ----------------
        if True:
            for j in SLOT_ORDER:
                po = psp.tile([P, 2, 512], F32, tag="ps", name="pvp")
                if j == 0:
                    for d in range(DT):
                        for h in range(2):
                            nc.tensor.matmul(
                                po[:, h, :],
                                lhsT=wxtb[:, d, :],
                                rhs=wvb[:, d, h * 512 : (h + 1) * 512],
                                start=(d == 0),
                                stop=(d == DT - 1),
                            )
                    oscale = OUT_BF
                else:
                    for dp in range(4):
                        for h in range(2):
                            nc.tensor.matmul(
                                po[:, h, :],
                                lhsT=wxt8[:, j, 2 * dp : 2 * dp + 2, :],
                                rhs=wv8[:, 2 * dp : 2 * dp + 2, h * 512 : (h + 1) * 512],
                                start=(dp == 0),
                                stop=(dp == 3),
                                perf_mode=DR,
                            )
                    oscale = OUT_F8
                ot = wtspool.tile([P, E], BF16, tag="ot", name="ot", bufs=4)
                if j == SLOT_ORDER[-1]:
                    nc.scalar.mul(ot[:, 0:256], po[:, 0, 0:256], oscale)
                    nc.vector.tensor_scalar_mul(ot[:, 256:512], po[:, 0, 256:512], oscale)
                    nc.scalar.mul(ot[:, 512:768], po[:, 1, 0:256], oscale)
                    nc.vector.tensor_scalar_mul(ot[:, 768:1024], po[:, 1, 256:512], oscale)
                else:
                    nc.scalar.mul(ot[:, 0:512], po[:, 0, :], oscale)
                    nc.vector.tensor_scalar_mul(ot[:, 512:1024], po[:, 1, :], oscale)
                q0 = QCOL[j]
                nc.sync.dma_start(out_d[q0 : q0 + P, :], ot[:])

    nc.compile()
    return nc


_NC_CACHE = None


def _get_nc():
    global _NC_CACHE
    if _NC_CACHE is None:
        _NC_CACHE = build_kernel()
    return _NC_CACHE


def _q8(a, scale):
    return np.clip(a * scale, -240.0, 240.0).astype(f8t)


def _pack_inputs(x, Wq, Wk, Wv):
    """Host-side relayout, weight folding, and quantization."""
    M = (Wq.T.astype(np.float64) @ Wk.astype(np.float64)).astype(np.float32)
    # packed as lhsT slices: [p, j_t, d, j_local] = M[d*128+p, j_t*128+j_local]
    mp = M.reshape(DT, P, DT, P).transpose(1, 2, 0, 3)
    m8 = np.ascontiguousarray(_q8(mp, S_M))
    mb = np.ascontiguousarray(mp.astype(bft))
    # Wv^T packed d-outer: [p, d, e] = Wv[e, d*128+p]
    wvp = Wv.reshape(E, DT, P).transpose(2, 1, 0)
    wv8 = np.ascontiguousarray(_q8(wvp, S_V))
    wvb = np.ascontiguousarray(wvp.astype(bft))

    def packmask(blocks):
        m = np.zeros((NSLOT, P, KCH), np.float32)
        for j, blk in enumerate(blocks):
            cc = np.arange(KCH)[None, :] + (CJ[j] - 1) * KCH
            rr = np.arange(P)[:, None] + blk * P
            m[j] = np.where(cc <= rr, 0.0, MASK_VAL)
        return np.ascontiguousarray(m.transpose(1, 0, 2).astype(bft))

    masks = [packmask(QBLOCKS[0]), packmask(QBLOCKS[1])]

    in_maps = []
    for c in range(N_CORES):
        b, h = divmod(c, 2)
        xb = x[b]  # [S, D]
        xt = xb.reshape(S, DT, P).transpose(2, 1, 0)       # [p, d, s]
        xnat = xb.reshape(S // P, P, D).transpose(1, 0, 2)  # [p, kb, d]
        # query rows: slots 1..7 first, slot 0 last
        rows = np.concatenate(
            [np.arange(QBLOCKS[h][j] * P, (QBLOCKS[h][j] + 1) * P)
             for j in list(range(1, NSLOT)) + [0]]
        )
        xqt = xb[rows].reshape(SQ, DT, P).transpose(2, 1, 0)  # [p, d, q]
        in_maps.append(
            {
                "xq8": np.ascontiguousarray(_q8(xqt, S_X)),
                "xqb": np.ascontiguousarray(xqt[:, :, 7 * P :].astype(bft)),
                "m8": m8,
                "mb": mb,
                "xt8": np.ascontiguousarray(_q8(xt, S_X)),
                "xtb": np.ascontiguousarray(xt[:, :, 0:KCH].astype(bft)),
                "xn8": np.ascontiguousarray(_q8(xnat, S_X)),
                "xnb": np.ascontiguousarray(xnat[:, 0:2, :].astype(bft)),
                "wv8": wv8,
                "wvb": wvb,
                "masks": masks[h],
            }
        )
    return in_maps


def kernel(x, Wq, Wk, Wv, _spmd_kwargs=None, _results_out=None):
    x = np.asarray(x, dtype=np.float32)
    Wq = np.asarray(Wq, dtype=np.float32)
    Wk = np.asarray(Wk, dtype=np.float32)
    Wv = np.asarray(Wv, dtype=np.float32)
    assert x.shape == (B, S, D)

    nc = _get_nc()
    in_maps = _pack_inputs(x, Wq, Wk, Wv)
    res = run_bass_kernel_spmd(
        nc, in_maps, list(range(N_CORES)), **(_spmd_kwargs or {})
    )
    if _results_out is not None:
        _results_out.append(res)

    out = np.empty((B, S, E), np.float32)
    for c in range(N_CORES):
        b, h = divmod(c, 2)
        o = res.results[c]["out"].astype(np.float32)
        for j in range(NSLOT):
            blk = QBLOCKS[h][j]
            q0 = QCOL[j]
            out[b, blk * P : (blk + 1) * P, :] = o[q0 : q0 + P, :]
    return out
